# revision 5
# baseline (speedup 1.0000x reference)
"""Trainium2 Bass kernel for nn_HADL_60593398612570.

The reference computes, per (batch, channel) row of x[B=128, L=720, C=862]:
  mean-subtract -> Haar lowpass (720->360) -> DCT-II/n -> @A[360,30] -> @B[30,720]
  -> +bias +mean -> output [128, 720, 862].

Everything is linear in x, and rank-limited by A/B, so it collapses to
  out[b, :, c] = x[b, :, c] @ P1a @ Ba + bias
with P1a [720, 31] = [Haar^T @ D^T @ A / 360 | ones/720] and
Ba [31, 720] = [B ; (1 - colsum(P1) @ B)] (the extra rank-1 handles the mean).

Sharding: data-parallel over batch, 16 batches per core x 8 cores.
Per core: stage 1  T[31,862]  = P1a^T @ x[b]   (contract L=720)
          stage 2  out[720,862] = Ba^T @ T + bias (contract rank=31)
All fp32 (exact).
"""

import math
import os

import numpy as np

import concourse.bass as bass
import concourse.bacc as bacc
import concourse.tile as tile
from concourse import mybir
from concourse.bass_utils import run_bass_kernel_spmd

B_FULL = 128
N_CORES = 8
B_PER = B_FULL // N_CORES  # 16
L = 720          # input seq len (contraction dim of stage 1)
C = 862          # channels (moving/free dim)
R = 31           # rank 30 + 1 mean column
NK = 6           # L tiles: 5x128 + 80
NP = 6           # output-seq tiles: 5x128 + 80
CH = 431         # C split in two chunks (fp32 moving max 512)
F32 = mybir.dt.float32


def _ktile(i):
    return 128 if i < 5 else L - 5 * 128  # 80 for the last tile


def _build_nc():
    nc = bacc.Bacc(
        "TRN2", target_bir_lowering=False, debug=False, num_devices=N_CORES
    )
    x = nc.dram_tensor("x", [B_PER, L, C], F32, kind="ExternalInput")
    p1a = nc.dram_tensor("p1a", [L, R], F32, kind="ExternalInput")
    ba = nc.dram_tensor("ba", [R, L], F32, kind="ExternalInput")
    biasc = nc.dram_tensor("biasc", [128, NP], F32, kind="ExternalInput")
    out = nc.dram_tensor("out", [B_PER, L, C], F32, kind="ExternalOutput")

    with tile.TileContext(nc) as tc:
        with (
            tc.tile_pool(name="consts", bufs=1) as consts,
            tc.tile_pool(name="xin", bufs=12) as xin,
            tc.tile_pool(name="tsb", bufs=2) as tsb,
            tc.tile_pool(name="osb", bufs=3) as osb,
            tc.tile_pool(name="tpsum", bufs=2, space="PSUM") as tpsum,
            tc.tile_pool(name="opsum", bufs=4, space="PSUM") as opsum,
        ):
            # one-time constant loads
            p1a_t = []
            for k in range(NK):
                kk = _ktile(k)
                t = consts.tile([kk, R], F32, tag=f"p1a{k}")
                nc.gpsimd.dma_start(out=t[:], in_=p1a[k * 128 : k * 128 + kk, :])
                p1a_t.append(t)
            ba_sb = consts.tile([R, L], F32, tag="ba")
            nc.gpsimd.dma_start(out=ba_sb[:], in_=ba[:])
            bias_sb = consts.tile([128, NP], F32, tag="bias")
            nc.gpsimd.dma_start(out=bias_sb[:], in_=biasc[:])

            for b in range(B_PER):
                # ---- load x[b] as 6 l-tiles [kk, 862]
                xt = []
                for k in range(NK):
                    kk = _ktile(k)
                    t = xin.tile([kk, C], F32, tag="x")
                    nc.gpsimd.dma_start(
                        out=t[:], in_=x[b, k * 128 : k * 128 + kk, :]
                    )
                    xt.append(t)

                # ---- stage 1: T[31, 862] = P1a^T @ x[b]
                tp = tpsum.tile([R, 2, 512], F32)
                for ch in range(2):
                    for k in range(NK):
                        kk = _ktile(k)
                        nc.tensor.matmul(
                            tp[:, ch, :CH],
                            p1a_t[k][:],
                            xt[k][:, ch * CH : (ch + 1) * CH],
                            start=(k == 0),
                            stop=(k == NK - 1),
                        )
                ts = tsb.tile([R, 2, CH], F32)
                nc.vector.tensor_copy(ts[:], tp[:, :, :CH])

                # ---- stage 2: out[b] = Ba^T @ T + bias
                for p in range(NP):
                    pp = _ktile(p)
                    ob = osb.tile([128, C], F32, tag="ob")
                    for ch in range(2):
                        op = opsum.tile([128, 512], F32, tag="op")
                        nc.tensor.matmul(
                            op[:pp, :CH],
                            ba_sb[:, p * 128 : p * 128 + pp],
                            ts[:, ch, :],
                            start=True,
                            stop=True,
                        )
                        dst = ob[:pp, ch * CH : (ch + 1) * CH]
                        if (p + ch) % 2 == 0:
                            nc.vector.tensor_scalar_add(
                                dst, op[:pp, :CH], bias_sb[:pp, p : p + 1]
                            )
                        else:
                            nc.scalar.add(
                                dst, op[:pp, :CH], bias_sb[:pp, p : p + 1]
                            )
                    nc.gpsimd.dma_start(
                        out=out[b, p * 128 : p * 128 + pp, :], in_=ob[:pp, :]
                    )
    nc.compile()
    return nc


_NC_CACHE = None


def _get_nc():
    global _NC_CACHE
    if _NC_CACHE is None:
        _NC_CACHE = _build_nc()
    return _NC_CACHE


def _host_weights(A, B, bvec):
    """Fold Haar + DCT + A + B + mean handling into P1a [720,31], Ba [31,720]."""
    n = 360
    k = np.arange(n)[:, None]
    m = np.arange(n)[None, :]
    D = np.sqrt(2.0 / n) * np.cos(np.pi * (m + 0.5) * k / n)
    D[0, :] *= 1.0 / np.sqrt(2.0)
    Ht = np.zeros((L, n))
    t = np.arange(n)
    Ht[2 * t, t] = 1.0 / math.sqrt(2.0)
    Ht[2 * t + 1, t] = 1.0 / math.sqrt(2.0)
    P1 = Ht @ D.T @ A.astype(np.float64) / n                # [720, 30]
    v = np.ones(L) - P1.sum(axis=0) @ B.astype(np.float64)  # [720]
    P1a = np.concatenate([P1, np.full((L, 1), 1.0 / L)], axis=1)
    Ba = np.concatenate([B.astype(np.float64), v[None, :]], axis=0)
    biasc = np.zeros((128, NP))
    bb = np.asarray(bvec, dtype=np.float64)
    for j in range(NP):
        jj = _ktile(j)
        biasc[:jj, j] = bb[j * 128 : j * 128 + jj]
    return (
        np.ascontiguousarray(P1a, dtype=np.float32),
        np.ascontiguousarray(Ba, dtype=np.float32),
        np.ascontiguousarray(biasc, dtype=np.float32),
    )


def _run(inputs, trace=False, **kw):
    x = np.asarray(inputs["x"], dtype=np.float32)
    P1a, Ba, biasc = _host_weights(
        np.asarray(inputs["A"]), np.asarray(inputs["B"]), np.asarray(inputs["b"])
    )
    nc = _get_nc()
    in_maps = [
        {
            "x": np.ascontiguousarray(x[i * B_PER : (i + 1) * B_PER]),
            "p1a": P1a,
            "ba": Ba,
            "biasc": biasc,
        }
        for i in range(N_CORES)
    ]
    res = run_bass_kernel_spmd(nc, in_maps, list(range(N_CORES)), trace=trace, **kw)
    out = np.concatenate([res.results[i]["out"] for i in range(N_CORES)], axis=0)
    return out, res


def kernel(**inputs):
    out, _ = _run(inputs, trace=bool(int(os.environ.get("KERNEL_TRACE", "0"))))
    return out
